# Initial kernel scaffold
#
"""Trainium2 Bass kernel for sparse (top-k) multi-headed attention.

Problem shapes (hardcoded):
  x, source: [B=4, D=256, N=M=2048] f32
  Wq/Wk/Wv/Wm: [256, 256], bq/bk/bv/bm: [256], k=32 (top-k), H=4 heads, dim=64.

Sharding: 8 cores; core c handles batch b=c//2 and head pair hp=c%2
(heads 2hp, 2hp+1).  Channel c of D maps to (d, h) = (c//4, c%4) per the
reference reshape(B, dim, H, N).  The host reorders each core's 128
channels head-major/d-major so each head occupies 64 contiguous SBUF
partitions.  Each core returns its partial merge
  part = Wm[:, ch].T? -> out_part[o, n] = sum_{i in ch} Wm[o, i] * merged[i, n]
and the host sums the two partials per batch and adds bm.

Top-k on device: e = exp(scores/8) (monotonic), 4 rounds of DVE max +
match_replace(imm=0) mutate a copy of e zeroing the top-32 entries; then
p_unnorm = e - mutated selects exactly the top-32 exps. den comes free via
scalar_tensor_tensor accum_out.
"""

import os
import sys

import numpy as np

for _p in ("/opt/trn_rl_repo",):
    if _p not in sys.path and os.path.isdir(_p):
        sys.path.insert(0, _p)

import concourse.bass as bass
import concourse.mybir as mybir
import concourse.tile as tile
from concourse.bass_utils import run_bass_kernel_spmd
from concourse.masks import make_identity

B, D, N, M = 4, 256, 2048, 2048
H = 4
DIM = D // H  # 64
P = 128
NT = N // P  # n-tiles of 128 rows
MT = M // P  # m-tiles of 128 cols
SCALE = 1.0 / float(np.sqrt(DIM))  # 0.125
N_CORES = 8

FP = mybir.dt.float32
A = mybir.AluOpType


def build_program(k: int) -> bass.Bass:
    nrounds = (k + 7) // 8
    rem = k - (nrounds - 1) * 8  # valid slots in the last round (1..8)

    nc = bass.Bass(
        "TRN2",
        target_bir_lowering=False,
        debug=False,
        enable_asserts=True,
        num_devices=N_CORES,
    )

    # DRAM parameters (per-core shards, prepared by the host)
    xb = nc.dram_tensor("xb", [D, N], FP, kind="ExternalInput").ap()
    src = nc.dram_tensor("src", [D, M], FP, kind="ExternalInput").ap()
    wqT = nc.dram_tensor("wqT", [D, P], FP, kind="ExternalInput").ap()
    wkT = nc.dram_tensor("wkT", [D, P], FP, kind="ExternalInput").ap()
    wvT = nc.dram_tensor("wvT", [D, P], FP, kind="ExternalInput").ap()
    wmT = nc.dram_tensor("wmT", [P, D], FP, kind="ExternalInput").ap()
    bqv = nc.dram_tensor("bq", [P, 1], FP, kind="ExternalInput").ap()
    bkv = nc.dram_tensor("bk", [P, 1], FP, kind="ExternalInput").ap()
    bvv = nc.dram_tensor("bv", [P, 1], FP, kind="ExternalInput").ap()
    part = nc.dram_tensor("part", [D, N], FP, kind="ExternalOutput").ap()

    from contextlib import ExitStack

    with tile.TileContext(nc) as tc, ExitStack() as ctx:
        consts = ctx.enter_context(tc.tile_pool(name="consts", bufs=1))
        wpool = ctx.enter_context(tc.tile_pool(name="w", bufs=1))
        xpool = ctx.enter_context(tc.tile_pool(name="x", bufs=1))
        qkvp = ctx.enter_context(tc.tile_pool(name="qkv", bufs=1))
        vtp = ctx.enter_context(tc.tile_pool(name="vt", bufs=1))

        identity = consts.tile([P, P], FP)
        make_identity(nc, identity)

        # ---- load weights / biases / activations ----
        w_tiles = {}
        for name, ap in (("wq", wqT), ("wk", wkT), ("wv", wvT)):
            t0 = wpool.tile([P, P], FP, tag=name + "0")
            t1 = wpool.tile([P, P], FP, tag=name + "1")
            nc.sync.dma_start(out=t0[:], in_=ap[0:P, :])
            nc.sync.dma_start(out=t1[:], in_=ap[P : 2 * P, :])
            w_tiles[name] = (t0, t1)
        wm_sb = wpool.tile([P, D], FP, tag="wm")
        nc.sync.dma_start(out=wm_sb[:], in_=wmT[:, :])
        b_tiles = {}
        for name, ap in (("bq", bqv), ("bk", bkv), ("bv", bvv)):
            t = wpool.tile([P, 1], FP, tag=name)
            nc.sync.dma_start(out=t[:], in_=ap[:, :])
            b_tiles[name] = t

        x_sb = [xpool.tile([P, N], FP, tag=f"x{i}") for i in range(2)]
        s_sb = [xpool.tile([P, M], FP, tag=f"s{i}") for i in range(2)]
        for i in range(2):
            nc.sync.dma_start(out=x_sb[i][:], in_=xb[i * P : (i + 1) * P, :])
            nc.sync.dma_start(out=s_sb[i][:], in_=src[i * P : (i + 1) * P, :])

        # ---- QKV projections: out[ch, n] = sum_i wT[i, ch] * in[i, n] + b[ch]
        q_sb = qkvp.tile([P, N], FP, tag="q")
        k_sb = qkvp.tile([P, M], FP, tag="k")
        v_sb = qkvp.tile([P, M], FP, tag="v")
        with tc.tile_pool(name="prps", bufs=2, space="PSUM") as prps:
            for (wname, bname, ins, out_sb) in (
                ("wq", "bq", x_sb, q_sb),
                ("wk", "bk", s_sb, k_sb),
                ("wv", "bv", s_sb, v_sb),
            ):
                w0, w1 = w_tiles[wname]
                bt = b_tiles[bname]
                for nf in range(4):
                    sl = slice(nf * 512, (nf + 1) * 512)
                    pp = prps.tile([P, 512], FP, tag="pp")
                    nc.tensor.matmul(
                        pp[:], lhsT=w0[:], rhs=ins[0][:, sl], start=True, stop=False
                    )
                    nc.tensor.matmul(
                        pp[:], lhsT=w1[:], rhs=ins[1][:, sl], start=False, stop=True
                    )
                    nc.vector.tensor_scalar(
                        out=out_sb[:, sl], in0=pp[:], scalar1=bt[:], scalar2=None,
                        op0=A.add,
                    )

        # ---- vT per head: vT_sb[h] cols (mt*64:(mt+1)*64) = v_h[:, mt*128:...].T
        vT_sb = [vtp.tile([P, MT * DIM], FP, tag=f"vT{h}") for h in range(2)]
        with tc.tile_pool(name="vtps", bufs=2, space="PSUM") as vtps:
            for h in range(2):
                hs = slice(h * DIM, (h + 1) * DIM)
                for mt in range(MT):
                    tp = vtps.tile([P, DIM], FP, tag="vtp")
                    nc.tensor.transpose(
                        tp[:], v_sb[hs, mt * P : (mt + 1) * P], identity[0:DIM, 0:DIM]
                    )
                    nc.scalar.activation(
                        out=vT_sb[h][:, mt * DIM : (mt + 1) * DIM], in_=tp[:],
                        func=mybir.ActivationFunctionType.Copy,
                    )

        # ---- main loop ----
        epool = ctx.enter_context(tc.tile_pool(name="e", bufs=2))
        scp = ctx.enter_context(tc.tile_pool(name="scr", bufs=2))
        ppool = ctx.enter_context(tc.tile_pool(name="p", bufs=2))
        m8p = ctx.enter_context(tc.tile_pool(name="m8", bufs=8))
        dpool = ctx.enter_context(tc.tile_pool(name="den", bufs=4))
        ptp = ctx.enter_context(tc.tile_pool(name="pt", bufs=4))
        mgp = ctx.enter_context(tc.tile_pool(name="mg", bufs=2))
        sps = ctx.enter_context(tc.tile_pool(name="sps", bufs=3, space="PSUM"))
        tps = ctx.enter_context(tc.tile_pool(name="tps", bufs=2, space="PSUM"))
        avps = ctx.enter_context(tc.tile_pool(name="avps", bufs=2, space="PSUM"))
        mgps = ctx.enter_context(tc.tile_pool(name="mgps", bufs=1, space="PSUM"))

        for nt in range(NT):
            n0 = nt * P
            mg_sb = mgp.tile([P, P], FP, tag="mg")
            for h in range(2):
                hs = slice(h * DIM, (h + 1) * DIM)
                # scores -> exp
                e = epool.tile([P, M], FP, tag="e")
                for mf in range(4):
                    sl = slice(mf * 512, (mf + 1) * 512)
                    sp = sps.tile([P, 512], FP, tag="sp")
                    nc.tensor.matmul(
                        sp[:], lhsT=q_sb[hs, n0 : n0 + P], rhs=k_sb[hs, sl],
                        start=True, stop=True,
                    )
                    nc.scalar.activation(
                        out=e[:, sl], in_=sp[:],
                        func=mybir.ActivationFunctionType.Exp, scale=float(SCALE),
                    )
                # top-k: zero out top-k entries of a copy of e
                scratch = scp.tile([P, M], FP, tag="scratch")
                src_t = e
                for r in range(nrounds):
                    m8 = m8p.tile([P, 8], FP, tag="m8")
                    nc.vector.max(out=m8[:], in_=src_t[:])
                    if r == nrounds - 1 and rem < 8:
                        nc.vector.memset(m8[:, rem:], 0.0)
                    nc.vector.match_replace(
                        out=scratch[:], in_to_replace=m8[:], in_values=src_t[:],
                        imm_value=0.0,
                    )
                    src_t = scratch
                # p_unnorm = e - scratch (top-k exps, 0 elsewhere); den = row sum
                p = ppool.tile([P, M], FP, tag="p")
                den = dpool.tile([P, 1], FP, tag="den")
                nc.gpsimd.scalar_tensor_tensor(
                    out=p[:], in0=e[:], scalar=0.0, in1=scratch[:],
                    op0=A.bypass, op1=A.subtract, accum_out=den[:],
                )
                rden = dpool.tile([P, 1], FP, tag="rden")
                nc.vector.reciprocal(rden[:], den[:])
                nc.gpsimd.tensor_scalar(
                    out=p[:], in0=p[:], scalar1=rden[:], scalar2=None, op0=A.mult
                )
                # AV: out_av[d, n] = sum_m v[d, m] * p[n, m]
                av = avps.tile([DIM, P], FP, tag="av")
                for mt in range(MT):
                    tp = tps.tile([P, P], FP, tag="ptp")
                    nc.tensor.transpose(
                        tp[:], p[:, mt * P : (mt + 1) * P], identity[:]
                    )
                    pT = ptp.tile([P, P], FP, tag="pT")
                    nc.scalar.activation(
                        out=pT[:], in_=tp[:], func=mybir.ActivationFunctionType.Copy
                    )
                    nc.tensor.matmul(
                        av[:], lhsT=vT_sb[h][:, mt * DIM : (mt + 1) * DIM], rhs=pT[:],
                        start=(mt == 0), stop=(mt == MT - 1),
                    )
                nc.scalar.activation(
                    out=mg_sb[hs, :], in_=av[:],
                    func=mybir.ActivationFunctionType.Copy,
                )
            # partial merge for this n block: [256 out channels] x [128 n]
            for oh in range(2):
                mm = mgps.tile([P, P], FP, tag="mm")
                nc.tensor.matmul(
                    mm[:], lhsT=wm_sb[:, oh * P : (oh + 1) * P], rhs=mg_sb[:],
                    start=True, stop=True,
                )
                nc.sync.dma_start(
                    out=part[oh * P : (oh + 1) * P, n0 : n0 + P], in_=mm[:]
                )

    return nc


_PROGRAM_CACHE: dict[int, bass.Bass] = {}
LAST_RESULTS = None


def _channel_order(hp: int) -> list[int]:
    # head-major, d-major within head: channels of head h are {4d + h}
    return [4 * d + 2 * hp + j for j in (0, 1) for d in range(DIM)]


def make_in_maps(x, source, Wq, bq, Wk, bk, Wv, bv, Wm):
    in_maps = []
    for c in range(N_CORES):
        b = c // 2
        hp = c % 2
        ch = _channel_order(hp)
        in_maps.append(
            {
                "xb": np.ascontiguousarray(x[b], dtype=np.float32),
                "src": np.ascontiguousarray(source[b], dtype=np.float32),
                "wqT": np.ascontiguousarray(Wq[ch, :].T, dtype=np.float32),
                "wkT": np.ascontiguousarray(Wk[ch, :].T, dtype=np.float32),
                "wvT": np.ascontiguousarray(Wv[ch, :].T, dtype=np.float32),
                "wmT": np.ascontiguousarray(Wm[:, ch].T, dtype=np.float32),
                "bq": np.ascontiguousarray(bq[ch].reshape(P, 1), dtype=np.float32),
                "bk": np.ascontiguousarray(bk[ch].reshape(P, 1), dtype=np.float32),
                "bv": np.ascontiguousarray(bv[ch].reshape(P, 1), dtype=np.float32),
            }
        )
    return in_maps


def kernel(x, source, Wq, bq, Wk, bk, Wv, bv, Wm, bm, k):
    global LAST_RESULTS
    k = int(k)
    x = np.asarray(x, dtype=np.float32)
    source = np.asarray(source, dtype=np.float32)
    nc = _PROGRAM_CACHE.get(k)
    if nc is None:
        nc = build_program(k)
        _PROGRAM_CACHE[k] = nc
    in_maps = make_in_maps(x, source, Wq, bq, Wk, bk, Wv, bv, Wm)
    res = run_bass_kernel_spmd(nc, in_maps, list(range(N_CORES)))
    LAST_RESULTS = res
    out = np.zeros((B, D, N), dtype=np.float32)
    for c in range(N_CORES):
        out[c // 2] += res.results[c]["part"]
    out += np.asarray(bm, dtype=np.float32)[None, :, None]
    return out


# revision 23
# speedup vs baseline: 1.0230x; 1.0230x over previous
"""Trainium2 Bass kernel for sparse (top-k) multi-headed attention.

Problem shapes (hardcoded):
  x, source: [B=4, D=256, N=M=2048] f32
  Wq/Wk/Wv/Wm: [256, 256], bq/bk/bv/bm: [256], k=32 (top-k), H=4 heads, dim=64.

Sharding: 8 cores; core c handles batch b=c//2 and head pair hp=c%2
(heads 2hp, 2hp+1).  Channel c of D maps to (d, h) = (c//4, c%4) per the
reference reshape(B, dim, H, N).  The host reorders each core's 128
channels head-major/d-major so each head occupies 64 contiguous SBUF
partitions.  Each core returns its partial merge
  part = Wm[:, ch].T? -> out_part[o, n] = sum_{i in ch} Wm[o, i] * merged[i, n]
and the host sums the two partials per batch and adds bm.

Top-k on device: e = exp(scores/8) (monotonic), 4 rounds of DVE max +
match_replace(imm=0) mutate a copy of e zeroing the top-32 entries; then
p_unnorm = e - mutated selects exactly the top-32 exps. den comes free via
scalar_tensor_tensor accum_out.
"""

import os
import sys

import ml_dtypes
import numpy as np

for _p in ("/opt/trn_rl_repo",):
    if _p not in sys.path and os.path.isdir(_p):
        sys.path.insert(0, _p)

import concourse.bass as bass
import concourse.mybir as mybir
import concourse.tile as tile
from concourse.bass_utils import run_bass_kernel_spmd
from concourse.masks import make_identity

B, D, N, M = 4, 256, 2048, 2048
H = 4
DIM = D // H  # 64
P = 128
NT = N // P  # n-tiles of 128 rows
MT = M // P  # m-tiles of 128 cols
SCALE = 1.0 / float(np.sqrt(DIM))  # 0.125
N_CORES = 8

FP = mybir.dt.float32
A = mybir.AluOpType



def _legalize_sync_waits(bir: dict) -> dict:
    """Split multi-wait instructions: walrus codegen allows only ONE sync wait
    per engine instruction (PE is HW-decoded; ACT/CTRL structs are just as
    limited).  Insert single-wait NoOps on the same engine immediately before
    any instruction carrying more than one wait; each NoOp takes one wait, the
    original keeps the last wait plus its updates."""
    nid = [0]
    for fn in bir["functions"]:
        for blk in fn["blocks"]:
            out = []
            for ins in blk["instructions"]:
                si = ins.get("sync_info")
                waits = (si or {}).get("on_wait") or []
                if len(waits) > 1:
                    for w in waits[:-1]:
                        nid[0] += 1
                        out.append(
                            {
                                "engine": ins["engine"],
                                "ins": [],
                                "name": f"{ins['name']}-sw{nid[0]}",
                                "opcode": "NoOp",
                                "outs": [],
                                "sync_info": {"on_update": [], "on_wait": [w]},
                            }
                        )
                    si["on_wait"] = [waits[-1]]
                out.append(ins)
            blk["instructions"] = out
    return bir


def build_program(k: int) -> bass.Bass:
    nrounds = (k + 7) // 8
    rem = k - (nrounds - 1) * 8  # valid slots in the last round (1..8)

    nc = bass.Bass(
        "TRN2",
        target_bir_lowering=False,
        debug=False,
        enable_asserts=True,
        num_devices=N_CORES,
    )

    # DRAM parameters (per-core shards, prepared by the host)
    xb = nc.dram_tensor("xb", [D, N], FP, kind="ExternalInput").ap()
    src = nc.dram_tensor("src", [D, M], FP, kind="ExternalInput").ap()
    wqT = nc.dram_tensor("wqT", [D, P], FP, kind="ExternalInput").ap()
    wkT = nc.dram_tensor("wkT", [D, P], FP, kind="ExternalInput").ap()
    wvT = nc.dram_tensor("wvT", [D, P], FP, kind="ExternalInput").ap()
    wmT = nc.dram_tensor("wmT", [P, D], FP, kind="ExternalInput").ap()
    bqv = nc.dram_tensor("bq", [P, 1], FP, kind="ExternalInput").ap()
    bkv = nc.dram_tensor("bk", [P, 1], FP, kind="ExternalInput").ap()
    bvv = nc.dram_tensor("bv", [P, 1], FP, kind="ExternalInput").ap()
    part = nc.dram_tensor("part", [D, N], FP, kind="ExternalOutput").ap()

    from contextlib import ExitStack

    with tile.TileContext(nc) as tc, ExitStack() as ctx:
        consts = ctx.enter_context(tc.tile_pool(name="consts", bufs=1))
        wpool = ctx.enter_context(tc.tile_pool(name="w", bufs=1))
        qkvp = ctx.enter_context(tc.tile_pool(name="qkv", bufs=1))
        vtp = ctx.enter_context(tc.tile_pool(name="vt", bufs=1))
        xpool_cm = tc.tile_pool(name="x", bufs=1)
        xpool = xpool_cm.__enter__()

        identity = consts.tile([P, P], FP)
        make_identity(nc, identity)

        # ---- load weights / biases / activations ----
        w_tiles = {}
        for name, ap in (("wq", wqT), ("wk", wkT), ("wv", wvT)):
            t0 = wpool.tile([P, P], FP, tag=name + "0")
            t1 = wpool.tile([P, P], FP, tag=name + "1")
            nc.sync.dma_start(out=t0[:], in_=ap[0:P, :])
            nc.sync.dma_start(out=t1[:], in_=ap[P : 2 * P, :])
            w_tiles[name] = (t0, t1)
        wm_sb = wpool.tile([P, D], FP, tag="wm")
        nc.sync.dma_start(out=wm_sb[:], in_=wmT[:, :])
        b_tiles = {}
        for name, ap in (("bq", bqv), ("bk", bkv), ("bv", bvv)):
            t = wpool.tile([P, 1], FP, tag=name)
            nc.sync.dma_start(out=t[:], in_=ap[:, :])
            b_tiles[name] = t

        x_sb = [xpool.tile([P, N], FP, tag=f"x{i}", name=f"x{i}") for i in range(2)]
        s_sb = [xpool.tile([P, M], FP, tag=f"s{i}", name=f"s{i}") for i in range(2)]
        for i in range(2):
            nc.sync.dma_start(out=x_sb[i][:], in_=xb[i * P : (i + 1) * P, :])
            nc.sync.dma_start(out=s_sb[i][:], in_=src[i * P : (i + 1) * P, :])

        # Persistent PSUM pools for the whole kernel (exactly 8 banks total);
        # never released, so banks are never recycled across phases (bank
        # recycling creates cross-engine waits on PE instructions, which are
        # HW-decoded and carry at most ONE sync wait).
        sps = ctx.enter_context(tc.tile_pool(name="sps", bufs=4, space="PSUM"))
        tps = ctx.enter_context(tc.tile_pool(name="tps", bufs=1, space="PSUM"))
        avps = ctx.enter_context(tc.tile_pool(name="avps", bufs=2, space="PSUM"))
        mgps = ctx.enter_context(tc.tile_pool(name="mgps", bufs=1, space="PSUM"))

        # Absorb every DMA-completion semaphore (and the gpsimd-built
        # identity) into PE's observed clock: one tiny single-wait matmul per
        # loaded tile, so no later PE instruction needs a second fresh wait.
        all_loaded = (
            [w_tiles[n][i] for n in ("wq", "wk", "wv") for i in range(2)]
            + [wm_sb]
            + [b_tiles[n] for n in ("bq", "bk", "bv")]
            + x_sb
            + s_sb
            + [identity]
        )
        junk = tps.tile([P, P], FP, tag="ptp", name="junk")
        for i, t in enumerate(all_loaded):
            nc.tensor.matmul(
                junk[0:1, i : i + 1], lhsT=t[:, 0:1], rhs=t[:, 0:1],
                start=True, stop=True, skip_group_check=True,
            )
        # Read the junk tile on ACT so any later PE instruction recycling this
        # PSUM slot waits on the Activation sem (its one allowed wait).
        junk_sink = consts.tile([1, len(all_loaded)], FP, name="junk_sink")
        nc.scalar.activation(
            out=junk_sink[:], in_=junk[0:1, 0 : len(all_loaded)],
            func=mybir.ActivationFunctionType.Copy,
        )

        # ---- QKV projections: out[ch, n] = sum_i wT[i, ch] * in[i, n] + b[ch]
        q_sb = qkvp.tile([P, N], FP, tag="q")
        k_sb = qkvp.tile([P, M], FP, tag="k")
        v_sb = qkvp.tile([P, M], FP, tag="v")
        for (wname, bname, ins, out_sb) in (
            ("wq", "bq", x_sb, q_sb),
            ("wk", "bk", s_sb, k_sb),
            ("wv", "bv", s_sb, v_sb),
        ):
            w0, w1 = w_tiles[wname]
            bt = b_tiles[bname]
            for nf in range(4):
                sl = slice(nf * 512, (nf + 1) * 512)
                pp = sps.tile([P, 512], FP, tag="sp", name="pp")
                nc.tensor.matmul(
                    pp[:], lhsT=w0[:], rhs=ins[0][:, sl], start=True, stop=False
                )
                nc.tensor.matmul(
                    pp[:], lhsT=w1[:], rhs=ins[1][:, sl], start=False, stop=True
                )
                nc.scalar.activation(
                    out=out_sb[:, sl], in_=pp[:],
                    func=mybir.ActivationFunctionType.Identity, bias=bt[:],
                )

        xpool_cm.__exit__(None, None, None)

        # ---- vT per head (bf16): vT_sb[h] cols mt*64.. = v_h[:, mt*128..].T
        BF = mybir.dt.bfloat16
        identity_bf = consts.tile([P, P], BF, name="identity_bf")
        nc.scalar.activation(
            out=identity_bf[:], in_=identity[:],
            func=mybir.ActivationFunctionType.Copy,
        )
        vT_sb = [
            vtp.tile([P, MT * DIM], BF, tag=f"vT{h}", name=f"vT{h}") for h in range(2)
        ]
        for h in range(2):
            hs = slice(h * DIM, (h + 1) * DIM)
            for mt in range(MT):
                tp = tps.tile([P, P], FP, tag="ptp", name="vtp")
                nc.tensor.transpose(
                    tp[0:P, 0:DIM], v_sb[hs, mt * P : (mt + 1) * P], identity[hs, hs]
                )
                nc.scalar.activation(
                    out=vT_sb[h][:, mt * DIM : (mt + 1) * DIM], in_=tp[0:P, 0:DIM],
                    func=mybir.ActivationFunctionType.Copy,
                )

        # ---- main loop: super-tiles of 512 query rows ----
        epool = ctx.enter_context(tc.tile_pool(name="e", bufs=3))
        scp = ctx.enter_context(tc.tile_pool(name="scr", bufs=3))
        ppool = ctx.enter_context(tc.tile_pool(name="p", bufs=3))
        m8p = ctx.enter_context(tc.tile_pool(name="m8", bufs=8))
        dpool = ctx.enter_context(tc.tile_pool(name="den", bufs=8))
        pnp = ctx.enter_context(tc.tile_pool(name="pn", bufs=8))
        ptp = ctx.enter_context(tc.tile_pool(name="pt", bufs=4))
        mgp = ctx.enter_context(tc.tile_pool(name="mg", bufs=2))
        NEG = -1.0e30
        ST = 4  # n-tiles per super-tile

        for st in range(NT // ST):
            n0 = st * ST * P
            mg_sb = mgp.tile([P, ST * P], FP, tag="mg")
            for h in range(2):
                hs = slice(h * DIM, (h + 1) * DIM)
                pends = []
                for ntl in range(ST):
                    nn0 = n0 + ntl * P
                    # scores: raw fp32 in SBUF (exact, for top-k) + exp
                    e = epool.tile([P, M], FP, tag="e", name="e")
                    s_sb = ppool.tile([P, M], FP, tag="s_sb", name="s_sb")
                    for mf in range(4):
                        sl = slice(mf * 512, (mf + 1) * 512)
                        sp = sps.tile([P, 512], FP, tag="sp", name="sp")
                        nc.tensor.matmul(
                            sp[:], lhsT=q_sb[hs, nn0 : nn0 + P], rhs=k_sb[hs, sl],
                            start=True, stop=True,
                        )
                        nc.scalar.activation(
                            out=s_sb[:, sl], in_=sp[:],
                            func=mybir.ActivationFunctionType.Copy,
                        )
                        nc.scalar.activation(
                            out=e[:, sl], in_=sp[:],
                            func=mybir.ActivationFunctionType.Exp,
                            scale=float(SCALE),
                        )
                    # top-k on raw scores (hardware exp can flatten
                    # near-equal scores; raw compares match the reference)
                    scratch = scp.tile([P, M], FP, tag="scratch", name="scratch")
                    m32 = m8p.tile([P, 8 * nrounds], FP, tag="m32", name="m32")
                    src_t = s_sb
                    for r in range(nrounds):
                        m8 = m32[:, r * 8 : (r + 1) * 8]
                        nc.vector.max(out=m8, in_=src_t[:])
                        if r == nrounds - 1 and rem < 8:
                            nc.vector.memset(m8[:, rem:], NEG)
                        nc.vector.match_replace(
                            out=scratch[:], in_to_replace=m8, in_values=src_t[:],
                            imm_value=NEG,
                        )
                        src_t = scratch
                    # e_mask = exp(scale*scratch) == e except 0 at top-k spots
                    emk = scp.tile([P, M], FP, tag="emk", name="emk")
                    nc.scalar.activation(
                        out=emk[:], in_=scratch[:],
                        func=mybir.ActivationFunctionType.Exp, scale=float(SCALE),
                    )
                    p = ppool.tile([P, M], FP, tag="p", name="p")
                    nc.gpsimd.tensor_sub(p[:], e[:], emk[:])
                    pends.append((m32, p))
                # den-chain after ALL rounds of this head: the DVE engine is
                # in-order, so a reduce waiting on ACT's e32 exp would
                # head-of-line-block the next tile's max/match_replace rounds
                pns = []
                for (m32, p) in pends:
                    # den = sum(exp(scale * top-k scores)); same exp table
                    e32 = dpool.tile([P, 8 * nrounds], FP, tag="e32", name="e32")
                    nc.scalar.activation(
                        out=e32[:], in_=m32[:],
                        func=mybir.ActivationFunctionType.Exp, scale=float(SCALE),
                    )
                    den = dpool.tile([P, 1], FP, tag="den", name="den")
                    nc.vector.tensor_reduce(
                        out=den[:], in_=e32[:], axis=mybir.AxisListType.X, op=A.add
                    )
                    rden = dpool.tile([P, 1], FP, tag="rden", name="rden")
                    nc.vector.reciprocal(rden[:], den[:])
                    pn = pnp.tile([P, M], BF, tag="pn", name="pn")
                    nc.scalar.activation(
                        out=pn[:], in_=p[:],
                        func=mybir.ActivationFunctionType.Copy, scale=rden[:],
                    )
                    pns.append(pn)
                # transpose p (bf16) and AV: av[d, n] = sum_m v[d,m] p[n,m]
                av = avps.tile([DIM, ST * P], FP, tag="av", name="av")
                for mt in range(MT):
                    pT = ptp.tile([P, ST * P], BF, tag="pT", name="pT")
                    for ntl in range(ST):
                        # bf16 transpose via the DMA XBAR: SBUF->SBUF, frees
                        # both PE (transpose) and ACT (PSUM->SBUF copy)
                        nc.sync.dma_start(
                            out=pT[:, ntl * P : (ntl + 1) * P],
                            in_=pns[ntl][:, mt * P : (mt + 1) * P],
                            transpose=True,
                        )
                    nc.tensor.matmul(
                        av[:], lhsT=vT_sb[h][:, mt * DIM : (mt + 1) * DIM],
                        rhs=pT[:], start=(mt == 0), stop=(mt == MT - 1),
                    )
                nc.scalar.activation(
                    out=mg_sb[hs, :], in_=av[:],
                    func=mybir.ActivationFunctionType.Copy,
                )
            # partial merge: [256 out channels] x [512 n]
            for oh in range(2):
                mm = mgps.tile([P, ST * P], FP, tag="mm", name="mm")
                nc.tensor.matmul(
                    mm[:], lhsT=wm_sb[:, oh * P : (oh + 1) * P], rhs=mg_sb[:],
                    start=True, stop=True,
                )
                mo = mgp.tile([P, ST * P], FP, tag="mo", name="mo")
                nc.scalar.activation(
                    out=mo[:], in_=mm[:], func=mybir.ActivationFunctionType.Copy
                )
                nc.sync.dma_start(
                    out=part[oh * P : (oh + 1) * P, n0 : n0 + ST * P], in_=mo[:]
                )

    import json as _json

    d = _json.loads(nc.to_json_bytes())
    _legalize_sync_waits(d)
    blob = _json.dumps(d).encode()
    nc.to_json_bytes = lambda: blob  # shadow the method; bass2jax serializes via this
    return nc


_PROGRAM_CACHE: dict[int, object] = {}
LAST_RESULTS = None


def _channel_order(hp: int) -> list[int]:
    # head-major, d-major within head: channels of head h are {4d + h}
    return [4 * d + 2 * hp + j for j in (0, 1) for d in range(DIM)]


def make_in_maps(x, source, Wq, bq, Wk, bk, Wv, bv, Wm):
    in_maps = []
    for c in range(N_CORES):
        b = c // 2
        hp = c % 2
        ch = _channel_order(hp)
        in_maps.append(
            {
                "xb": np.ascontiguousarray(x[b], dtype=np.float32),
                "src": np.ascontiguousarray(source[b], dtype=np.float32),
                "wqT": np.ascontiguousarray(Wq[ch, :].T, dtype=np.float32),
                "wkT": np.ascontiguousarray(Wk[ch, :].T, dtype=np.float32),
                "wvT": np.ascontiguousarray(Wv[ch, :].T, dtype=np.float32),
                "wmT": np.ascontiguousarray(Wm[:, ch].T, dtype=np.float32),
                "bq": np.ascontiguousarray(bq[ch].reshape(P, 1), dtype=np.float32),
                "bk": np.ascontiguousarray(bk[ch].reshape(P, 1), dtype=np.float32),
                "bv": np.ascontiguousarray(bv[ch].reshape(P, 1), dtype=np.float32),
            }
        )
    return in_maps


class _CompiledProgram:
    """Builds the Bass program once and caches the jitted shard_map callable
    (mirrors the multi-core branch of bass2jax.run_bass_via_pjrt)."""

    def __init__(self, k: int):
        import jax
        from jax.sharding import Mesh, PartitionSpec
        from jax.experimental.shard_map import shard_map
        from concourse import bass2jax

        bass2jax.install_neuronx_cc_hook()
        nc = build_program(k)
        self.nc = nc
        import concourse.mybir as _mybir

        in_names, out_names, out_avals, zero_outs = [], [], [], []
        for alloc in nc.m.functions[0].allocations:
            if not isinstance(alloc, _mybir.MemoryLocationSet):
                continue
            name = alloc.memorylocations[0].name
            partition_name = (
                nc.partition_id_tensor.name if nc.partition_id_tensor else None
            )
            if alloc.kind == "ExternalInput":
                if name != partition_name:
                    in_names.append(name)
            elif alloc.kind == "ExternalOutput":
                out_names.append(name)
                shape = tuple(alloc.tensor_shape)
                dtype = _mybir.dt.np(alloc.dtype)
                out_avals.append(jax.core.ShapedArray(shape, dtype))
                zero_outs.append(np.zeros(shape, dtype))
        self.in_names = list(in_names)
        self.out_names = out_names
        n_params = len(in_names)
        n_outs = len(out_avals)
        in_names = in_names + out_names
        self.in_names = self.in_names[:n_params]
        donate = tuple(range(n_params, n_params + n_outs))
        self.zero_outs = zero_outs
        self.out_avals = out_avals

        partition_name = (
            nc.partition_id_tensor.name if nc.partition_id_tensor else None
        )
        if partition_name is not None:
            in_names = in_names + [partition_name]

        def _body(*args):
            operands = list(args)
            if partition_name is not None:
                operands.append(bass2jax.partition_id_tensor())
            outs = bass2jax._bass_exec_p.bind(
                *operands,
                out_avals=tuple(out_avals),
                in_names=tuple(in_names),
                out_names=tuple(out_names),
                lowering_input_output_aliases=(),
                sim_require_finite=True,
                sim_require_nnan=True,
                nc=nc,
            )
            return tuple(outs)

        devices = jax.devices()[:N_CORES]
        mesh = Mesh(np.asarray(devices), ("core",))
        in_specs = (PartitionSpec("core"),) * (n_params + n_outs)
        out_specs = (PartitionSpec("core"),) * len(out_names)
        self.sharded = jax.jit(
            shard_map(
                _body, mesh=mesh, in_specs=in_specs, out_specs=out_specs,
                check_rep=False,
            ),
            donate_argnums=donate,
            keep_unused=True,
        )
        self.jax = jax

    def run(self, in_maps):
        np_in = [
            np.concatenate([np.asarray(m[name]) for m in in_maps], axis=0)
            for name in self.in_names
        ]
        zeros = [
            np.zeros((N_CORES * z.shape[0], *z.shape[1:]), z.dtype)
            for z in self.zero_outs
        ]
        out_arrs = self.jax.block_until_ready(self.sharded(*np_in, *zeros))
        return [
            {
                name: np.asarray(out_arrs[i]).reshape(
                    N_CORES, *self.out_avals[i].shape
                )[c]
                for i, name in enumerate(self.out_names)
            }
            for c in range(N_CORES)
        ]


def _get_program(k: int) -> _CompiledProgram:
    prog = _PROGRAM_CACHE.get(k)
    if prog is None:
        prog = _CompiledProgram(k)
        _PROGRAM_CACHE[k] = prog
    return prog


def kernel(x, source, Wq, bq, Wk, bk, Wv, bv, Wm, bm, k):
    global LAST_RESULTS
    k = int(k)
    x = np.asarray(x, dtype=np.float32)
    source = np.asarray(source, dtype=np.float32)
    prog = _get_program(k)
    in_maps = make_in_maps(x, source, Wq, bq, Wk, bk, Wv, bv, Wm)
    results = prog.run(in_maps)
    LAST_RESULTS = results
    out = np.zeros((B, D, N), dtype=np.float32)
    for c in range(N_CORES):
        out[c // 2] += results[c]["part"]
    out += np.asarray(bm, dtype=np.float32)[None, :, None]
    return out


# revision 25
# speedup vs baseline: 1.4175x; 1.3856x over previous
"""Trainium2 Bass kernel for sparse (top-k) multi-headed attention.

Problem shapes (hardcoded):
  x, source: [B=4, D=256, N=M=2048] f32
  Wq/Wk/Wv/Wm: [256, 256], bq/bk/bv/bm: [256], k=32 (top-k), H=4 heads, dim=64.

Sharding: 8 cores; core c handles batch b=c//2 and head pair hp=c%2
(heads 2hp, 2hp+1).  Channel c of D maps to (d, h) = (c//4, c%4) per the
reference reshape(B, dim, H, N).  The host reorders each core's 128
channels head-major/d-major so each head occupies 64 contiguous SBUF
partitions.  Each core returns its partial merge
  part = Wm[:, ch].T? -> out_part[o, n] = sum_{i in ch} Wm[o, i] * merged[i, n]
and the host sums the two partials per batch and adds bm.

Top-k on device: e = exp(scores/8) (monotonic), 4 rounds of DVE max +
match_replace(imm=0) mutate a copy of e zeroing the top-32 entries; then
p_unnorm = e - mutated selects exactly the top-32 exps. den comes free via
scalar_tensor_tensor accum_out.
"""

import os
import sys

import ml_dtypes
import numpy as np

for _p in ("/opt/trn_rl_repo",):
    if _p not in sys.path and os.path.isdir(_p):
        sys.path.insert(0, _p)

import concourse.bass as bass
import concourse.mybir as mybir
import concourse.tile as tile
from concourse.bass_utils import run_bass_kernel_spmd
from concourse.masks import make_identity

B, D, N, M = 4, 256, 2048, 2048
H = 4
DIM = D // H  # 64
P = 128
NT = N // P  # n-tiles of 128 rows
MT = M // P  # m-tiles of 128 cols
SCALE = 1.0 / float(np.sqrt(DIM))  # 0.125
N_CORES = 8

FP = mybir.dt.float32
A = mybir.AluOpType



def _legalize_sync_waits(bir: dict) -> dict:
    """Split multi-wait instructions: walrus codegen allows only ONE sync wait
    per engine instruction (PE is HW-decoded; ACT/CTRL structs are just as
    limited).  Insert single-wait NoOps on the same engine immediately before
    any instruction carrying more than one wait; each NoOp takes one wait, the
    original keeps the last wait plus its updates."""
    nid = [0]
    for fn in bir["functions"]:
        for blk in fn["blocks"]:
            out = []
            for ins in blk["instructions"]:
                si = ins.get("sync_info")
                waits = (si or {}).get("on_wait") or []
                if len(waits) > 1:
                    for w in waits[:-1]:
                        nid[0] += 1
                        out.append(
                            {
                                "engine": ins["engine"],
                                "ins": [],
                                "name": f"{ins['name']}-sw{nid[0]}",
                                "opcode": "NoOp",
                                "outs": [],
                                "sync_info": {"on_update": [], "on_wait": [w]},
                            }
                        )
                    si["on_wait"] = [waits[-1]]
                out.append(ins)
            blk["instructions"] = out
    return bir


def build_program(k: int) -> bass.Bass:
    nrounds = (k + 7) // 8
    rem = k - (nrounds - 1) * 8  # valid slots in the last round (1..8)

    nc = bass.Bass(
        "TRN2",
        target_bir_lowering=False,
        debug=False,
        enable_asserts=True,
        num_devices=N_CORES,
    )

    # DRAM parameters (per-core shards, prepared by the host)
    xb = nc.dram_tensor("xb", [D, N], FP, kind="ExternalInput").ap()
    src = nc.dram_tensor("src", [D, M], FP, kind="ExternalInput").ap()
    wqT = nc.dram_tensor("wqT", [D, P], FP, kind="ExternalInput").ap()
    wkT = nc.dram_tensor("wkT", [D, P], FP, kind="ExternalInput").ap()
    wvT = nc.dram_tensor("wvT", [D, P], FP, kind="ExternalInput").ap()
    wmT = nc.dram_tensor("wmT", [P, D], FP, kind="ExternalInput").ap()
    bqv = nc.dram_tensor("bq", [P, 1], FP, kind="ExternalInput").ap()
    bkv = nc.dram_tensor("bk", [P, 1], FP, kind="ExternalInput").ap()
    bvv = nc.dram_tensor("bv", [P, 1], FP, kind="ExternalInput").ap()
    part = nc.dram_tensor("part", [D, N], FP, kind="ExternalOutput").ap()

    from contextlib import ExitStack

    with tile.TileContext(nc) as tc, ExitStack() as ctx:
        consts = ctx.enter_context(tc.tile_pool(name="consts", bufs=1))
        wpool = ctx.enter_context(tc.tile_pool(name="w", bufs=1))
        qkvp = ctx.enter_context(tc.tile_pool(name="qkv", bufs=1))
        vtp = ctx.enter_context(tc.tile_pool(name="vt", bufs=1))
        xpool_cm = tc.tile_pool(name="x", bufs=1)
        xpool = xpool_cm.__enter__()

        identity = consts.tile([P, P], FP)
        make_identity(nc, identity)

        # ---- load weights / biases / activations ----
        w_tiles = {}
        for name, ap in (("wq", wqT), ("wk", wkT), ("wv", wvT)):
            t0 = wpool.tile([P, P], FP, tag=name + "0")
            t1 = wpool.tile([P, P], FP, tag=name + "1")
            nc.sync.dma_start(out=t0[:], in_=ap[0:P, :])
            nc.sync.dma_start(out=t1[:], in_=ap[P : 2 * P, :])
            w_tiles[name] = (t0, t1)
        wm_sb = wpool.tile([P, D], FP, tag="wm")
        nc.sync.dma_start(out=wm_sb[:], in_=wmT[:, :])
        b_tiles = {}
        for name, ap in (("bq", bqv), ("bk", bkv), ("bv", bvv)):
            t = wpool.tile([P, 1], FP, tag=name)
            nc.sync.dma_start(out=t[:], in_=ap[:, :])
            b_tiles[name] = t

        x_sb = [xpool.tile([P, N], FP, tag=f"x{i}", name=f"x{i}") for i in range(2)]
        s_sb = [xpool.tile([P, M], FP, tag=f"s{i}", name=f"s{i}") for i in range(2)]
        for i in range(2):
            nc.sync.dma_start(out=x_sb[i][:], in_=xb[i * P : (i + 1) * P, :])
            nc.sync.dma_start(out=s_sb[i][:], in_=src[i * P : (i + 1) * P, :])

        # Persistent PSUM pools for the whole kernel (exactly 8 banks total);
        # never released, so banks are never recycled across phases (bank
        # recycling creates cross-engine waits on PE instructions, which are
        # HW-decoded and carry at most ONE sync wait).
        sps = ctx.enter_context(tc.tile_pool(name="sps", bufs=3, space="PSUM"))
        tps = ctx.enter_context(tc.tile_pool(name="tps", bufs=2, space="PSUM"))
        avps = ctx.enter_context(tc.tile_pool(name="avps", bufs=2, space="PSUM"))
        mgps = ctx.enter_context(tc.tile_pool(name="mgps", bufs=1, space="PSUM"))

        # Absorb every DMA-completion semaphore (and the gpsimd-built
        # identity) into PE's observed clock: one tiny single-wait matmul per
        # loaded tile, so no later PE instruction needs a second fresh wait.
        all_loaded = (
            [w_tiles[n][i] for n in ("wq", "wk", "wv") for i in range(2)]
            + [wm_sb]
            + [b_tiles[n] for n in ("bq", "bk", "bv")]
            + x_sb
            + s_sb
            + [identity]
        )
        junk = tps.tile([P, P], FP, tag="ptp", name="junk")
        for i, t in enumerate(all_loaded):
            nc.tensor.matmul(
                junk[0:1, i : i + 1], lhsT=t[:, 0:1], rhs=t[:, 0:1],
                start=True, stop=True, skip_group_check=True,
            )
        # Read the junk tile on ACT so any later PE instruction recycling this
        # PSUM slot waits on the Activation sem (its one allowed wait).
        junk_sink = consts.tile([1, len(all_loaded)], FP, name="junk_sink")
        nc.scalar.activation(
            out=junk_sink[:], in_=junk[0:1, 0 : len(all_loaded)],
            func=mybir.ActivationFunctionType.Copy,
        )

        # ---- QKV projections: out[ch, n] = sum_i wT[i, ch] * in[i, n] + b[ch]
        q_sb = qkvp.tile([P, N], FP, tag="q")
        k_sb = qkvp.tile([P, M], FP, tag="k")
        v_sb = qkvp.tile([P, M], FP, tag="v")
        for (wname, bname, ins, out_sb) in (
            ("wq", "bq", x_sb, q_sb),
            ("wk", "bk", s_sb, k_sb),
            ("wv", "bv", s_sb, v_sb),
        ):
            w0, w1 = w_tiles[wname]
            bt = b_tiles[bname]
            for nf in range(4):
                sl = slice(nf * 512, (nf + 1) * 512)
                pp = sps.tile([P, 512], FP, tag="sp", name="pp")
                nc.tensor.matmul(
                    pp[:], lhsT=w0[:], rhs=ins[0][:, sl], start=True, stop=False
                )
                nc.tensor.matmul(
                    pp[:], lhsT=w1[:], rhs=ins[1][:, sl], start=False, stop=True
                )
                nc.scalar.activation(
                    out=out_sb[:, sl], in_=pp[:],
                    func=mybir.ActivationFunctionType.Identity, bias=bt[:],
                )

        xpool_cm.__exit__(None, None, None)

        # ---- vT per head (bf16): vT_sb[h] cols mt*64.. = v_h[:, mt*128..].T
        BF = mybir.dt.bfloat16
        identity_bf = consts.tile([P, P], BF, name="identity_bf")
        nc.scalar.activation(
            out=identity_bf[:], in_=identity[:],
            func=mybir.ActivationFunctionType.Copy,
        )
        vT_sb = [
            vtp.tile([P, MT * DIM], BF, tag=f"vT{h}", name=f"vT{h}") for h in range(2)
        ]
        for h in range(2):
            hs = slice(h * DIM, (h + 1) * DIM)
            for mt in range(MT):
                tp = tps.tile([P, P], FP, tag="ptp", name="vtp")
                nc.tensor.transpose(
                    tp[0:P, 0:DIM], v_sb[hs, mt * P : (mt + 1) * P], identity[hs, hs]
                )
                nc.scalar.activation(
                    out=vT_sb[h][:, mt * DIM : (mt + 1) * DIM], in_=tp[0:P, 0:DIM],
                    func=mybir.ActivationFunctionType.Copy,
                )

        # ---- main loop: super-tiles of 512 query rows ----
        epool = ctx.enter_context(tc.tile_pool(name="e", bufs=2))
        scp = ctx.enter_context(tc.tile_pool(name="scr", bufs=4))
        ppool = ctx.enter_context(tc.tile_pool(name="p", bufs=3))
        m8p = ctx.enter_context(tc.tile_pool(name="m8", bufs=8))
        dpool = ctx.enter_context(tc.tile_pool(name="den", bufs=8))
        pnp = ctx.enter_context(tc.tile_pool(name="pn", bufs=8))
        ptp = ctx.enter_context(tc.tile_pool(name="pt", bufs=4))
        mgp = ctx.enter_context(tc.tile_pool(name="mg", bufs=2))
        NEG = -1.0e30
        ST = 4  # n-tiles per super-tile

        for st in range(NT // ST):
            n0 = st * ST * P
            mg_sb = mgp.tile([P, ST * P], FP, tag="mg")
            for h in range(2):
                hs = slice(h * DIM, (h + 1) * DIM)
                pends = []
                for ntl in range(ST):
                    nn0 = n0 + ntl * P
                    # scores: raw fp32 in SBUF (exact, for top-k) + exp
                    e = epool.tile([P, M], FP, tag="e", name="e")
                    s_sb = ppool.tile([P, M], FP, tag="s_sb", name="s_sb")
                    for mf in range(4):
                        sl = slice(mf * 512, (mf + 1) * 512)
                        sp = sps.tile([P, 512], FP, tag="sp", name="sp")
                        nc.tensor.matmul(
                            sp[:], lhsT=q_sb[hs, nn0 : nn0 + P], rhs=k_sb[hs, sl],
                            start=True, stop=True,
                        )
                        nc.scalar.activation(
                            out=s_sb[:, sl], in_=sp[:],
                            func=mybir.ActivationFunctionType.Copy,
                        )
                        nc.scalar.activation(
                            out=e[:, sl], in_=sp[:],
                            func=mybir.ActivationFunctionType.Exp,
                            scale=float(SCALE),
                        )
                    # top-k on raw scores (hardware exp can flatten
                    # near-equal scores; raw compares match the reference)
                    scratch = scp.tile([P, M], FP, tag="scratch", name="scratch")
                    m32 = m8p.tile([P, 8 * nrounds], FP, tag="m32", name="m32")
                    src_t = s_sb
                    for r in range(nrounds):
                        m8 = m32[:, r * 8 : (r + 1) * 8]
                        nc.vector.max(out=m8, in_=src_t[:])
                        if r == nrounds - 1 and rem < 8:
                            nc.vector.memset(m8[:, rem:], NEG)
                        nc.vector.match_replace(
                            out=scratch[:], in_to_replace=m8, in_values=src_t[:],
                            imm_value=NEG,
                        )
                        src_t = scratch
                    # e_mask = exp(scale*scratch) == e except 0 at top-k spots
                    emk = scp.tile([P, M], FP, tag="emk", name="emk")
                    nc.scalar.activation(
                        out=emk[:], in_=scratch[:],
                        func=mybir.ActivationFunctionType.Exp, scale=float(SCALE),
                    )
                    p = ppool.tile([P, M], FP, tag="p", name="p")
                    nc.gpsimd.tensor_sub(p[:], e[:], emk[:])
                    pends.append((m32, p))
                # den-chain after ALL rounds of this head: the DVE engine is
                # in-order, so a reduce waiting on ACT's e32 exp would
                # head-of-line-block the next tile's max/match_replace rounds
                pns = []
                for (m32, p) in pends:
                    # den = sum(exp(scale * top-k scores)); same exp table
                    e32 = dpool.tile([P, 8 * nrounds], FP, tag="e32", name="e32")
                    nc.scalar.activation(
                        out=e32[:], in_=m32[:],
                        func=mybir.ActivationFunctionType.Exp, scale=float(SCALE),
                    )
                    den = dpool.tile([P, 1], FP, tag="den", name="den")
                    nc.vector.tensor_reduce(
                        out=den[:], in_=e32[:], axis=mybir.AxisListType.X, op=A.add
                    )
                    rden = dpool.tile([P, 1], FP, tag="rden", name="rden")
                    nc.vector.reciprocal(rden[:], den[:])
                    pn = pnp.tile([P, M], BF, tag="pn", name="pn")
                    nc.scalar.activation(
                        out=pn[:], in_=p[:],
                        func=mybir.ActivationFunctionType.Copy, scale=rden[:],
                    )
                    pns.append(pn)
                # transpose p (bf16) and AV: av[d, n] = sum_m v[d,m] p[n,m]
                av = avps.tile([DIM, ST * P], FP, tag="av", name="av")
                for mt in range(MT):
                    pT = ptp.tile([P, ST * P], BF, tag="pT", name="pT")
                    tp = tps.tile([P, ST * P], BF, tag="ptp", name="tp")
                    for ntl in range(ST):
                        nc.tensor.transpose(
                            tp[:, ntl * P : (ntl + 1) * P],
                            pns[ntl][:, mt * P : (mt + 1) * P],
                            identity_bf[:],
                        )
                    nc.scalar.activation(
                        out=pT[:], in_=tp[:],
                        func=mybir.ActivationFunctionType.Copy,
                    )
                    nc.tensor.matmul(
                        av[:], lhsT=vT_sb[h][:, mt * DIM : (mt + 1) * DIM],
                        rhs=pT[:], start=(mt == 0), stop=(mt == MT - 1),
                    )
                nc.scalar.activation(
                    out=mg_sb[hs, :], in_=av[:],
                    func=mybir.ActivationFunctionType.Copy,
                )
            # partial merge: [256 out channels] x [512 n]
            for oh in range(2):
                mm = mgps.tile([P, ST * P], FP, tag="mm", name="mm")
                nc.tensor.matmul(
                    mm[:], lhsT=wm_sb[:, oh * P : (oh + 1) * P], rhs=mg_sb[:],
                    start=True, stop=True,
                )
                mo = mgp.tile([P, ST * P], FP, tag="mo", name="mo")
                nc.scalar.activation(
                    out=mo[:], in_=mm[:], func=mybir.ActivationFunctionType.Copy
                )
                nc.sync.dma_start(
                    out=part[oh * P : (oh + 1) * P, n0 : n0 + ST * P], in_=mo[:]
                )

    import json as _json

    d = _json.loads(nc.to_json_bytes())
    _legalize_sync_waits(d)
    blob = _json.dumps(d).encode()
    nc.to_json_bytes = lambda: blob  # shadow the method; bass2jax serializes via this
    return nc


_PROGRAM_CACHE: dict[int, object] = {}
LAST_RESULTS = None


def _channel_order(hp: int) -> list[int]:
    # head-major, d-major within head: channels of head h are {4d + h}
    return [4 * d + 2 * hp + j for j in (0, 1) for d in range(DIM)]


def make_in_maps(x, source, Wq, bq, Wk, bk, Wv, bv, Wm):
    in_maps = []
    for c in range(N_CORES):
        b = c // 2
        hp = c % 2
        ch = _channel_order(hp)
        in_maps.append(
            {
                "xb": np.ascontiguousarray(x[b], dtype=np.float32),
                "src": np.ascontiguousarray(source[b], dtype=np.float32),
                "wqT": np.ascontiguousarray(Wq[ch, :].T, dtype=np.float32),
                "wkT": np.ascontiguousarray(Wk[ch, :].T, dtype=np.float32),
                "wvT": np.ascontiguousarray(Wv[ch, :].T, dtype=np.float32),
                "wmT": np.ascontiguousarray(Wm[:, ch].T, dtype=np.float32),
                "bq": np.ascontiguousarray(bq[ch].reshape(P, 1), dtype=np.float32),
                "bk": np.ascontiguousarray(bk[ch].reshape(P, 1), dtype=np.float32),
                "bv": np.ascontiguousarray(bv[ch].reshape(P, 1), dtype=np.float32),
            }
        )
    return in_maps


class _CompiledProgram:
    """Builds the Bass program once and caches the jitted shard_map callable
    (mirrors the multi-core branch of bass2jax.run_bass_via_pjrt)."""

    def __init__(self, k: int):
        import jax
        from jax.sharding import Mesh, PartitionSpec
        from jax.experimental.shard_map import shard_map
        from concourse import bass2jax

        bass2jax.install_neuronx_cc_hook()
        nc = build_program(k)
        self.nc = nc
        import concourse.mybir as _mybir

        in_names, out_names, out_avals, zero_outs = [], [], [], []
        for alloc in nc.m.functions[0].allocations:
            if not isinstance(alloc, _mybir.MemoryLocationSet):
                continue
            name = alloc.memorylocations[0].name
            partition_name = (
                nc.partition_id_tensor.name if nc.partition_id_tensor else None
            )
            if alloc.kind == "ExternalInput":
                if name != partition_name:
                    in_names.append(name)
            elif alloc.kind == "ExternalOutput":
                out_names.append(name)
                shape = tuple(alloc.tensor_shape)
                dtype = _mybir.dt.np(alloc.dtype)
                out_avals.append(jax.core.ShapedArray(shape, dtype))
                zero_outs.append(np.zeros(shape, dtype))
        self.in_names = list(in_names)
        self.out_names = out_names
        n_params = len(in_names)
        n_outs = len(out_avals)
        in_names = in_names + out_names
        self.in_names = self.in_names[:n_params]
        donate = tuple(range(n_params, n_params + n_outs))
        self.zero_outs = zero_outs
        self.out_avals = out_avals

        partition_name = (
            nc.partition_id_tensor.name if nc.partition_id_tensor else None
        )
        if partition_name is not None:
            in_names = in_names + [partition_name]

        def _body(*args):
            operands = list(args)
            if partition_name is not None:
                operands.append(bass2jax.partition_id_tensor())
            outs = bass2jax._bass_exec_p.bind(
                *operands,
                out_avals=tuple(out_avals),
                in_names=tuple(in_names),
                out_names=tuple(out_names),
                lowering_input_output_aliases=(),
                sim_require_finite=True,
                sim_require_nnan=True,
                nc=nc,
            )
            return tuple(outs)

        devices = jax.devices()[:N_CORES]
        mesh = Mesh(np.asarray(devices), ("core",))
        in_specs = (PartitionSpec("core"),) * (n_params + n_outs)
        out_specs = (PartitionSpec("core"),) * len(out_names)
        self.sharded = jax.jit(
            shard_map(
                _body, mesh=mesh, in_specs=in_specs, out_specs=out_specs,
                check_rep=False,
            ),
            donate_argnums=donate,
            keep_unused=True,
        )
        self.jax = jax

    def run(self, in_maps):
        np_in = [
            np.concatenate([np.asarray(m[name]) for m in in_maps], axis=0)
            for name in self.in_names
        ]
        zeros = [
            np.zeros((N_CORES * z.shape[0], *z.shape[1:]), z.dtype)
            for z in self.zero_outs
        ]
        out_arrs = self.jax.block_until_ready(self.sharded(*np_in, *zeros))
        return [
            {
                name: np.asarray(out_arrs[i]).reshape(
                    N_CORES, *self.out_avals[i].shape
                )[c]
                for i, name in enumerate(self.out_names)
            }
            for c in range(N_CORES)
        ]


def _get_program(k: int) -> _CompiledProgram:
    prog = _PROGRAM_CACHE.get(k)
    if prog is None:
        prog = _CompiledProgram(k)
        _PROGRAM_CACHE[k] = prog
    return prog


def kernel(x, source, Wq, bq, Wk, bk, Wv, bv, Wm, bm, k):
    global LAST_RESULTS
    k = int(k)
    x = np.asarray(x, dtype=np.float32)
    source = np.asarray(source, dtype=np.float32)
    prog = _get_program(k)
    in_maps = make_in_maps(x, source, Wq, bq, Wk, bk, Wv, bv, Wm)
    results = prog.run(in_maps)
    LAST_RESULTS = results
    out = np.zeros((B, D, N), dtype=np.float32)
    for c in range(N_CORES):
        out[c // 2] += results[c]["part"]
    out += np.asarray(bm, dtype=np.float32)[None, :, None]
    return out


# revision 26
# speedup vs baseline: 1.5129x; 1.0673x over previous
"""Trainium2 Bass kernel for sparse (top-k) multi-headed attention.

Problem shapes (hardcoded):
  x, source: [B=4, D=256, N=M=2048] f32
  Wq/Wk/Wv/Wm: [256, 256], bq/bk/bv/bm: [256], k=32 (top-k), H=4 heads, dim=64.

Sharding: 8 cores; core c handles batch b=c//2 and head pair hp=c%2
(heads 2hp, 2hp+1).  Channel c of D maps to (d, h) = (c//4, c%4) per the
reference reshape(B, dim, H, N).  The host reorders each core's 128
channels head-major/d-major so each head occupies 64 contiguous SBUF
partitions.  Each core returns its partial merge
  part = Wm[:, ch].T? -> out_part[o, n] = sum_{i in ch} Wm[o, i] * merged[i, n]
and the host sums the two partials per batch and adds bm.

Top-k on device: e = exp(scores/8) (monotonic), 4 rounds of DVE max +
match_replace(imm=0) mutate a copy of e zeroing the top-32 entries; then
p_unnorm = e - mutated selects exactly the top-32 exps. den comes free via
scalar_tensor_tensor accum_out.
"""

import os
import sys

import ml_dtypes
import numpy as np

for _p in ("/opt/trn_rl_repo",):
    if _p not in sys.path and os.path.isdir(_p):
        sys.path.insert(0, _p)

import concourse.bass as bass
import concourse.mybir as mybir
import concourse.tile as tile
from concourse.bass_utils import run_bass_kernel_spmd
from concourse.masks import make_identity

B, D, N, M = 4, 256, 2048, 2048
H = 4
DIM = D // H  # 64
P = 128
NT = N // P  # n-tiles of 128 rows
MT = M // P  # m-tiles of 128 cols
SCALE = 1.0 / float(np.sqrt(DIM))  # 0.125
N_CORES = 8

FP = mybir.dt.float32
A = mybir.AluOpType



def _legalize_sync_waits(bir: dict) -> dict:
    """Split multi-wait instructions: walrus codegen allows only ONE sync wait
    per engine instruction (PE is HW-decoded; ACT/CTRL structs are just as
    limited).  Insert single-wait NoOps on the same engine immediately before
    any instruction carrying more than one wait; each NoOp takes one wait, the
    original keeps the last wait plus its updates."""
    nid = [0]
    for fn in bir["functions"]:
        for blk in fn["blocks"]:
            out = []
            for ins in blk["instructions"]:
                si = ins.get("sync_info")
                waits = (si or {}).get("on_wait") or []
                if len(waits) > 1:
                    for w in waits[:-1]:
                        nid[0] += 1
                        out.append(
                            {
                                "engine": ins["engine"],
                                "ins": [],
                                "name": f"{ins['name']}-sw{nid[0]}",
                                "opcode": "NoOp",
                                "outs": [],
                                "sync_info": {"on_update": [], "on_wait": [w]},
                            }
                        )
                    si["on_wait"] = [waits[-1]]
                out.append(ins)
            blk["instructions"] = out
    return bir


def build_program(k: int) -> bass.Bass:
    nrounds = (k + 7) // 8
    rem = k - (nrounds - 1) * 8  # valid slots in the last round (1..8)

    nc = bass.Bass(
        "TRN2",
        target_bir_lowering=False,
        debug=False,
        enable_asserts=True,
        num_devices=N_CORES,
    )

    # DRAM parameters (per-core shards, prepared by the host)
    xb = nc.dram_tensor("xb", [D, N], FP, kind="ExternalInput").ap()
    src = nc.dram_tensor("src", [D, M], FP, kind="ExternalInput").ap()
    wqT = nc.dram_tensor("wqT", [D, P], FP, kind="ExternalInput").ap()
    wkT = nc.dram_tensor("wkT", [D, P], FP, kind="ExternalInput").ap()
    wvT = nc.dram_tensor("wvT", [D, P], FP, kind="ExternalInput").ap()
    wmT = nc.dram_tensor("wmT", [P, D], FP, kind="ExternalInput").ap()
    bqv = nc.dram_tensor("bq", [P, 1], FP, kind="ExternalInput").ap()
    bkv = nc.dram_tensor("bk", [P, 1], FP, kind="ExternalInput").ap()
    bvv = nc.dram_tensor("bv", [P, 1], FP, kind="ExternalInput").ap()
    part = nc.dram_tensor("part", [D, N], FP, kind="ExternalOutput").ap()

    from contextlib import ExitStack

    with tile.TileContext(nc) as tc, ExitStack() as ctx:
        consts = ctx.enter_context(tc.tile_pool(name="consts", bufs=1))
        wpool = ctx.enter_context(tc.tile_pool(name="w", bufs=1))
        qkvp = ctx.enter_context(tc.tile_pool(name="qkv", bufs=1))
        vtp = ctx.enter_context(tc.tile_pool(name="vt", bufs=1))
        xpool_cm = tc.tile_pool(name="x", bufs=1)
        xpool = xpool_cm.__enter__()

        identity = consts.tile([P, P], FP)
        make_identity(nc, identity)

        # ---- load weights / biases / activations ----
        w_tiles = {}
        for name, ap in (("wq", wqT), ("wk", wkT), ("wv", wvT)):
            t0 = wpool.tile([P, P], FP, tag=name + "0")
            t1 = wpool.tile([P, P], FP, tag=name + "1")
            nc.sync.dma_start(out=t0[:], in_=ap[0:P, :])
            nc.sync.dma_start(out=t1[:], in_=ap[P : 2 * P, :])
            w_tiles[name] = (t0, t1)
        wm_sb = wpool.tile([P, D], FP, tag="wm")
        nc.sync.dma_start(out=wm_sb[:], in_=wmT[:, :])
        b_tiles = {}
        for name, ap in (("bq", bqv), ("bk", bkv), ("bv", bvv)):
            t = wpool.tile([P, 1], FP, tag=name)
            nc.sync.dma_start(out=t[:], in_=ap[:, :])
            b_tiles[name] = t

        x_sb = [xpool.tile([P, N], FP, tag=f"x{i}", name=f"x{i}") for i in range(2)]
        s_sb = [xpool.tile([P, M], FP, tag=f"s{i}", name=f"s{i}") for i in range(2)]
        for i in range(2):
            nc.sync.dma_start(out=x_sb[i][:], in_=xb[i * P : (i + 1) * P, :])
            nc.sync.dma_start(out=s_sb[i][:], in_=src[i * P : (i + 1) * P, :])

        # Persistent PSUM pools for the whole kernel (exactly 8 banks total);
        # never released, so banks are never recycled across phases (bank
        # recycling creates cross-engine waits on PE instructions, which are
        # HW-decoded and carry at most ONE sync wait).
        sps = ctx.enter_context(tc.tile_pool(name="sps", bufs=3, space="PSUM"))
        tps = ctx.enter_context(tc.tile_pool(name="tps", bufs=2, space="PSUM"))
        avps = ctx.enter_context(tc.tile_pool(name="avps", bufs=2, space="PSUM"))
        mgps = ctx.enter_context(tc.tile_pool(name="mgps", bufs=1, space="PSUM"))

        # Absorb every DMA-completion semaphore (and the gpsimd-built
        # identity) into PE's observed clock: one tiny single-wait matmul per
        # loaded tile, so no later PE instruction needs a second fresh wait.
        all_loaded = (
            [w_tiles[n][i] for n in ("wq", "wk", "wv") for i in range(2)]
            + [wm_sb]
            + [b_tiles[n] for n in ("bq", "bk", "bv")]
            + x_sb
            + s_sb
            + [identity]
        )
        junk = tps.tile([P, P], FP, tag="ptp", name="junk")
        for i, t in enumerate(all_loaded):
            nc.tensor.matmul(
                junk[0:1, i : i + 1], lhsT=t[:, 0:1], rhs=t[:, 0:1],
                start=True, stop=True, skip_group_check=True,
            )
        # Read the junk tile on ACT so any later PE instruction recycling this
        # PSUM slot waits on the Activation sem (its one allowed wait).
        junk_sink = consts.tile([1, len(all_loaded)], FP, name="junk_sink")
        nc.scalar.activation(
            out=junk_sink[:], in_=junk[0:1, 0 : len(all_loaded)],
            func=mybir.ActivationFunctionType.Copy,
        )

        # ---- QKV projections: out[ch, n] = sum_i wT[i, ch] * in[i, n] + b[ch]
        q_sb = qkvp.tile([P, N], FP, tag="q")
        k_sb = qkvp.tile([P, M], FP, tag="k")
        v_sb = qkvp.tile([P, M], FP, tag="v")
        for (wname, bname, ins, out_sb) in (
            ("wq", "bq", x_sb, q_sb),
            ("wk", "bk", s_sb, k_sb),
            ("wv", "bv", s_sb, v_sb),
        ):
            w0, w1 = w_tiles[wname]
            bt = b_tiles[bname]
            for nf in range(4):
                sl = slice(nf * 512, (nf + 1) * 512)
                pp = sps.tile([P, 512], FP, tag="sp", name="pp")
                nc.tensor.matmul(
                    pp[:], lhsT=w0[:], rhs=ins[0][:, sl], start=True, stop=False
                )
                nc.tensor.matmul(
                    pp[:], lhsT=w1[:], rhs=ins[1][:, sl], start=False, stop=True
                )
                nc.scalar.activation(
                    out=out_sb[:, sl], in_=pp[:],
                    func=mybir.ActivationFunctionType.Identity, bias=bt[:],
                )

        xpool_cm.__exit__(None, None, None)

        # ---- vT per head (bf16): vT_sb[h] cols mt*64.. = v_h[:, mt*128..].T
        BF = mybir.dt.bfloat16
        identity_bf = consts.tile([P, P], BF, name="identity_bf")
        nc.scalar.activation(
            out=identity_bf[:], in_=identity[:],
            func=mybir.ActivationFunctionType.Copy,
        )
        vT_sb = [
            vtp.tile([P, MT * DIM], BF, tag=f"vT{h}", name=f"vT{h}") for h in range(2)
        ]
        for h in range(2):
            hs = slice(h * DIM, (h + 1) * DIM)
            for mt in range(MT):
                tp = tps.tile([P, P], FP, tag="ptp", name="vtp")
                nc.tensor.transpose(
                    tp[0:P, 0:DIM], v_sb[hs, mt * P : (mt + 1) * P], identity[hs, hs]
                )
                nc.scalar.activation(
                    out=vT_sb[h][:, mt * DIM : (mt + 1) * DIM], in_=tp[0:P, 0:DIM],
                    func=mybir.ActivationFunctionType.Copy,
                )

        # ---- main loop: super-tiles of 512 query rows ----
        epool = ctx.enter_context(tc.tile_pool(name="e", bufs=3))
        scp = ctx.enter_context(tc.tile_pool(name="scr", bufs=3))
        ppool = ctx.enter_context(tc.tile_pool(name="p", bufs=3))
        m8p = ctx.enter_context(tc.tile_pool(name="m8", bufs=8))
        dpool = ctx.enter_context(tc.tile_pool(name="den", bufs=8))
        pnp = ctx.enter_context(tc.tile_pool(name="pn", bufs=8))
        ptp = ctx.enter_context(tc.tile_pool(name="pt", bufs=4))
        mgp = ctx.enter_context(tc.tile_pool(name="mg", bufs=2))
        NEG = -1.0e30
        ST = 4  # n-tiles per super-tile

        for st in range(NT // ST):
            n0 = st * ST * P
            mg_sb = mgp.tile([P, ST * P], FP, tag="mg")
            for h in range(2):
                hs = slice(h * DIM, (h + 1) * DIM)
                pends = []
                for ntl in range(ST):
                    nn0 = n0 + ntl * P
                    # scores: raw fp32 in SBUF (exact, for top-k) + exp
                    e = epool.tile([P, M], FP, tag="e", name="e")
                    s_sb = ppool.tile([P, M], FP, tag="s_sb", name="s_sb")
                    for mf in range(4):
                        sl = slice(mf * 512, (mf + 1) * 512)
                        sp = sps.tile([P, 512], FP, tag="sp", name="sp")
                        nc.tensor.matmul(
                            sp[:], lhsT=q_sb[hs, nn0 : nn0 + P], rhs=k_sb[hs, sl],
                            start=True, stop=True,
                        )
                        nc.scalar.activation(
                            out=s_sb[:, sl], in_=sp[:],
                            func=mybir.ActivationFunctionType.Copy,
                        )
                        nc.scalar.activation(
                            out=e[:, sl], in_=sp[:],
                            func=mybir.ActivationFunctionType.Exp,
                            scale=float(SCALE),
                        )
                    # top-k on raw scores (hardware exp can flatten
                    # near-equal scores; raw compares match the reference)
                    scratch = scp.tile([P, M], FP, tag="scratch", name="scratch")
                    m32 = m8p.tile([P, 8 * nrounds], FP, tag="m32", name="m32")
                    src_t = s_sb
                    for r in range(nrounds):
                        m8 = m32[:, r * 8 : (r + 1) * 8]
                        nc.vector.max(out=m8, in_=src_t[:])
                        if r == nrounds - 1 and rem < 8:
                            nc.vector.memset(m8[:, rem:], NEG)
                        nc.vector.match_replace(
                            out=scratch[:], in_to_replace=m8, in_values=src_t[:],
                            imm_value=NEG,
                        )
                        src_t = scratch
                    # e_mask = exp(scale*scratch) == e except 0 at top-k spots
                    emk = scp.tile([P, M], FP, tag="emk", name="emk")
                    nc.scalar.activation(
                        out=emk[:], in_=scratch[:],
                        func=mybir.ActivationFunctionType.Exp, scale=float(SCALE),
                    )
                    p = ppool.tile([P, M], FP, tag="p", name="p")
                    nc.gpsimd.tensor_sub(p[:], e[:], emk[:])
                    pends.append((m32, p))
                # den-chain after ALL rounds of this head: the DVE engine is
                # in-order, so a reduce waiting on ACT's e32 exp would
                # head-of-line-block the next tile's max/match_replace rounds
                pns = []
                for (m32, p) in pends:
                    # den = sum(exp(scale * top-k scores)); same exp table
                    e32 = dpool.tile([P, 8 * nrounds], FP, tag="e32", name="e32")
                    nc.scalar.activation(
                        out=e32[:], in_=m32[:],
                        func=mybir.ActivationFunctionType.Exp, scale=float(SCALE),
                    )
                    den = dpool.tile([P, 1], FP, tag="den", name="den")
                    nc.vector.tensor_reduce(
                        out=den[:], in_=e32[:], axis=mybir.AxisListType.X, op=A.add
                    )
                    rden = dpool.tile([P, 1], FP, tag="rden", name="rden")
                    nc.vector.reciprocal(rden[:], den[:])
                    pn = pnp.tile([P, M], BF, tag="pn", name="pn")
                    nc.scalar.activation(
                        out=pn[:], in_=p[:],
                        func=mybir.ActivationFunctionType.Copy, scale=rden[:],
                    )
                    pns.append(pn)
                # transpose p (bf16) and AV: av[d, n] = sum_m v[d,m] p[n,m]
                av = avps.tile([DIM, ST * P], FP, tag="av", name="av")
                for mt in range(MT):
                    pT = ptp.tile([P, ST * P], BF, tag="pT", name="pT")
                    tp = tps.tile([P, ST * P], BF, tag="ptp", name="tp")
                    for ntl in range(ST):
                        nc.tensor.transpose(
                            tp[:, ntl * P : (ntl + 1) * P],
                            pns[ntl][:, mt * P : (mt + 1) * P],
                            identity_bf[:],
                        )
                    nc.scalar.activation(
                        out=pT[:], in_=tp[:],
                        func=mybir.ActivationFunctionType.Copy,
                    )
                    nc.tensor.matmul(
                        av[:], lhsT=vT_sb[h][:, mt * DIM : (mt + 1) * DIM],
                        rhs=pT[:], start=(mt == 0), stop=(mt == MT - 1),
                    )
                nc.scalar.activation(
                    out=mg_sb[hs, :], in_=av[:],
                    func=mybir.ActivationFunctionType.Copy,
                )
            # partial merge: [256 out channels] x [512 n]
            for oh in range(2):
                mm = mgps.tile([P, ST * P], FP, tag="mm", name="mm")
                nc.tensor.matmul(
                    mm[:], lhsT=wm_sb[:, oh * P : (oh + 1) * P], rhs=mg_sb[:],
                    start=True, stop=True,
                )
                mo = mgp.tile([P, ST * P], FP, tag="mo", name="mo")
                nc.scalar.activation(
                    out=mo[:], in_=mm[:], func=mybir.ActivationFunctionType.Copy
                )
                nc.sync.dma_start(
                    out=part[oh * P : (oh + 1) * P, n0 : n0 + ST * P], in_=mo[:]
                )

    import json as _json

    d = _json.loads(nc.to_json_bytes())
    _legalize_sync_waits(d)
    blob = _json.dumps(d).encode()
    nc.to_json_bytes = lambda: blob  # shadow the method; bass2jax serializes via this
    return nc


_PROGRAM_CACHE: dict[int, object] = {}
LAST_RESULTS = None


def _channel_order(hp: int) -> list[int]:
    # head-major, d-major within head: channels of head h are {4d + h}
    return [4 * d + 2 * hp + j for j in (0, 1) for d in range(DIM)]


def make_in_maps(x, source, Wq, bq, Wk, bk, Wv, bv, Wm):
    in_maps = []
    for c in range(N_CORES):
        b = c // 2
        hp = c % 2
        ch = _channel_order(hp)
        in_maps.append(
            {
                "xb": np.ascontiguousarray(x[b], dtype=np.float32),
                "src": np.ascontiguousarray(source[b], dtype=np.float32),
                "wqT": np.ascontiguousarray(Wq[ch, :].T, dtype=np.float32),
                "wkT": np.ascontiguousarray(Wk[ch, :].T, dtype=np.float32),
                "wvT": np.ascontiguousarray(Wv[ch, :].T, dtype=np.float32),
                "wmT": np.ascontiguousarray(Wm[:, ch].T, dtype=np.float32),
                "bq": np.ascontiguousarray(bq[ch].reshape(P, 1), dtype=np.float32),
                "bk": np.ascontiguousarray(bk[ch].reshape(P, 1), dtype=np.float32),
                "bv": np.ascontiguousarray(bv[ch].reshape(P, 1), dtype=np.float32),
            }
        )
    return in_maps


class _CompiledProgram:
    """Builds the Bass program once and caches the jitted shard_map callable
    (mirrors the multi-core branch of bass2jax.run_bass_via_pjrt)."""

    def __init__(self, k: int):
        import jax
        from jax.sharding import Mesh, PartitionSpec
        from jax.experimental.shard_map import shard_map
        from concourse import bass2jax

        bass2jax.install_neuronx_cc_hook()
        nc = build_program(k)
        self.nc = nc
        import concourse.mybir as _mybir

        in_names, out_names, out_avals, zero_outs = [], [], [], []
        for alloc in nc.m.functions[0].allocations:
            if not isinstance(alloc, _mybir.MemoryLocationSet):
                continue
            name = alloc.memorylocations[0].name
            partition_name = (
                nc.partition_id_tensor.name if nc.partition_id_tensor else None
            )
            if alloc.kind == "ExternalInput":
                if name != partition_name:
                    in_names.append(name)
            elif alloc.kind == "ExternalOutput":
                out_names.append(name)
                shape = tuple(alloc.tensor_shape)
                dtype = _mybir.dt.np(alloc.dtype)
                out_avals.append(jax.core.ShapedArray(shape, dtype))
                zero_outs.append(np.zeros(shape, dtype))
        self.in_names = list(in_names)
        self.out_names = out_names
        n_params = len(in_names)
        n_outs = len(out_avals)
        in_names = in_names + out_names
        self.in_names = self.in_names[:n_params]
        donate = tuple(range(n_params, n_params + n_outs))
        self.zero_outs = zero_outs
        self.out_avals = out_avals

        partition_name = (
            nc.partition_id_tensor.name if nc.partition_id_tensor else None
        )
        if partition_name is not None:
            in_names = in_names + [partition_name]

        def _body(*args):
            operands = list(args)
            if partition_name is not None:
                operands.append(bass2jax.partition_id_tensor())
            outs = bass2jax._bass_exec_p.bind(
                *operands,
                out_avals=tuple(out_avals),
                in_names=tuple(in_names),
                out_names=tuple(out_names),
                lowering_input_output_aliases=(),
                sim_require_finite=True,
                sim_require_nnan=True,
                nc=nc,
            )
            return tuple(outs)

        devices = jax.devices()[:N_CORES]
        mesh = Mesh(np.asarray(devices), ("core",))
        in_specs = (PartitionSpec("core"),) * (n_params + n_outs)
        out_specs = (PartitionSpec("core"),) * len(out_names)
        self.sharded = jax.jit(
            shard_map(
                _body, mesh=mesh, in_specs=in_specs, out_specs=out_specs,
                check_rep=False,
            ),
            donate_argnums=donate,
            keep_unused=True,
        )
        self.jax = jax

    def run(self, in_maps):
        np_in = [
            np.concatenate([np.asarray(m[name]) for m in in_maps], axis=0)
            for name in self.in_names
        ]
        zeros = [
            np.zeros((N_CORES * z.shape[0], *z.shape[1:]), z.dtype)
            for z in self.zero_outs
        ]
        out_arrs = self.jax.block_until_ready(self.sharded(*np_in, *zeros))
        return [
            {
                name: np.asarray(out_arrs[i]).reshape(
                    N_CORES, *self.out_avals[i].shape
                )[c]
                for i, name in enumerate(self.out_names)
            }
            for c in range(N_CORES)
        ]


def _get_program(k: int) -> _CompiledProgram:
    prog = _PROGRAM_CACHE.get(k)
    if prog is None:
        prog = _CompiledProgram(k)
        _PROGRAM_CACHE[k] = prog
    return prog


def kernel(x, source, Wq, bq, Wk, bk, Wv, bv, Wm, bm, k):
    global LAST_RESULTS
    k = int(k)
    x = np.asarray(x, dtype=np.float32)
    source = np.asarray(source, dtype=np.float32)
    prog = _get_program(k)
    in_maps = make_in_maps(x, source, Wq, bq, Wk, bk, Wv, bv, Wm)
    results = prog.run(in_maps)
    LAST_RESULTS = results
    out = np.zeros((B, D, N), dtype=np.float32)
    for c in range(N_CORES):
        out[c // 2] += results[c]["part"]
    out += np.asarray(bm, dtype=np.float32)[None, :, None]
    return out
